# revision 39
# baseline (speedup 1.0000x reference)
"""AutoWeightedCELoss Trainium2 kernel (fp8 DoubleRow edition).

Computes mean(class_w[label] * CE(cls_score, label) * boundary_weight) for
B=8, C=4, H=W=512, data-parallel over 8 NeuronCores (1 sample per core).

Math (per sample):
  boundary weight: pix = CON + sa*Ga + sb*Gb + sab*Gab, where the label's 2
  bits give +-1 "spin" maps sa, sb, sab=sa*sb, CON is position-only
  (host-precomputed) and G_m = sum_k c'_k box_k(m), c'_k = -1/(4(k^2-1)),
  k = 3,5,9,17,33.

  box pipeline (transposed (w,h) layout so vertical shifts ride free axis):
    Cv^T[w,h'] = sum_h m[h,w] U[h,h']      PE f16 matmul, triangular U
    Dv_k[w,h'] = Cv^T(:,h'+p) - Cv^T(:,h'-p-1)
        one full-width DVE sub per (map,scale) into an fp8e4 tile; edge
        handling via a padded Cv^T tile (left pad 0, right pad Cv(511)).
        Dv values are small integers (|.|<=33) -> essentially exact in fp8.
    G^T[w',h'] = sum_k sum_w (2^9 c'_k band_k)[w,w'] Dv_k[w,h']
        PE fp8e4 DoubleRow matmuls: each instruction computes TWO band
        contractions at 0.5 cycles/row, PSUM-accumulated per w'-chunk.
        The 2^9 scaling keeps band values in fp8 normal range; it is
        undone in the f16 combine (v_m = 2^-9 * spT_m * G^T_m).

  CE in f16 (scores host-cast to f16): nll = ln(sum_c exp s_c) - s_label.
  Transposes (spins, nll) go through the DMA xbar (dma_start_transpose),
  not the PE. Reductions: T* = per-partition accum columns of
  q = nll^T*pix^T against spin maps; N* accumulated for free on the spin
  build ops. Host combines: loss = sum_c W_c S_c / N.
"""

import sys

sys.path.insert(0, "/opt/trn_rl_repo")

import numpy as np

import concourse.bacc as bacc
import concourse.mybir as mybir
from concourse import bass
from concourse.tile import TileContext
from concourse.bass_utils import run_bass_kernel_spmd

try:
    from ml_dtypes import float8_e4m3fn as E4M3
except ImportError:  # pragma: no cover
    E4M3 = None

F32 = mybir.dt.float32
F16 = mybir.dt.float16
F8 = mybir.dt.float8e4
I32 = mybir.dt.int32
I8 = mybir.dt.int8
OP = mybir.AluOpType
ACTF = mybir.ActivationFunctionType
DR = mybir.MatmulPerfMode.DoubleRow

B, C, H, W = 8, 4, 512, 512
P = 128          # partitions
NT = H // P      # 4 h-tiles (and w-tiles)
WID = NT * W     # 2048 wide-tile free size
N_CORES = 8
KS = [3, 5, 9, 17, 33]
PADS = [1, 2, 4, 8, 16]
NK = len(KS)
CP = [-1.0 / (4.0 * (k * k - 1)) for k in KS]   # -c_k/4
SC = 9           # bands carry 2^SC * c'_k; undone in the combine
CVW = 546        # padded Cv row: 17 left pad + 512 + 17 right pad
LP = 17          # left pad columns (zeros)

# accumulator columns (T reductions; N-sums are host-side class counts)
A_T0, A_TA, A_TBQ, A_TABQ = 0, 1, 2, 3
NACC = 4


def _host_constants():
    h = np.arange(H, dtype=np.float64)
    U = np.triu(np.ones((H, H), dtype=np.float16))            # U[h,h'] = h<=h'
    M = np.zeros((NK, W, W), dtype=np.float32)
    con = np.ones((H, W), dtype=np.float64)
    for i, k in enumerate(KS):
        p = PADS[i]
        d = np.abs(np.arange(W)[:, None] - np.arange(W)[None, :])
        M[i] = (d <= p).astype(np.float32) * np.float32(CP[i] * 2.0**SC)
        rc = np.minimum(h + p, H - 1) - np.maximum(h - p, 0) + 1  # rows in win
        con += 0.75 * rc[:, None] * rc[None, :] / (k * k - 1)
    return U, M.astype(E4M3), con.astype(np.float16)


def _wide(dram_ap):
    """(H, W) dram tensor -> [P, NT, W] access pattern (h-tiles stacked)."""
    return dram_ap.rearrange("(t p) w -> p t w", p=P)


def _w3(tile_ap):
    """[P, NT*W] sbuf tile -> [P, NT, W] view to pair with _wide()."""
    return tile_ap.rearrange("p (t w) -> p t w", t=NT)


def build_nc(debug=False):
    nc = bacc.Bacc(None, target_bir_lowering=False, debug=True)

    score = nc.dram_tensor("score", [C, H, W], F16, kind="ExternalInput")
    label = nc.dram_tensor("label", [H, W], I8, kind="ExternalInput")
    u16d = nc.dram_tensor("u16", [H, H], F16, kind="ExternalInput")
    m8d = nc.dram_tensor("m8", [NK, W, W], F8, kind="ExternalInput")
    cond = nc.dram_tensor("con", [H, W], F16, kind="ExternalInput")
    accd = nc.dram_tensor("acc", [P, NACC], F32, kind="ExternalOutput")
    if debug:
        pix_dbg = nc.dram_tensor("pix_dbg", [W, H], F16, kind="ExternalOutput")
        nll_dbg = nc.dram_tensor("nll_dbg", [H, W], F16, kind="ExternalOutput")
        gt_dbg = nc.dram_tensor("gt_dbg", [3, W, H], F32, kind="ExternalOutput")

    with TileContext(nc) as tc:
        with (
            tc.tile_pool(name="sb", bufs=1) as sb,
            tc.tile_pool(name="ps", bufs=1, space="PSUM") as ps,
        ):
            # ---- input DMAs, spread across the three DMA-capable seqs ----
            # sync: label (critical path head)
            lbl = sb.tile([P, WID], I8, tag="lbl")
            for t in range(NT):
                nc.sync.dma_start(_w3(lbl[:])[:, t : t + 1, :],
                                  _wide(label[:])[:, t : t + 1, :])
            # scalar: U then CON
            u16 = sb.tile([P, WID], F16, tag="u16")
            for t in range(NT):
                nc.scalar.dma_start(_w3(u16[:])[:, t : t + 1, :],
                                    _wide(u16d[:])[:, t : t + 1, :])
            pix = sb.tile([P, WID], F16, tag="pix")   # starts as CON^T (=CON)
            for hlf in range(2):
                nc.scalar.dma_start(
                    _w3(pix[:])[:, 2 * hlf : 2 * hlf + 2, :],
                    _wide(cond[:])[:, 2 * hlf : 2 * hlf + 2, :])
            # gpsimd: bands, scores, then transposed spins
            m8 = sb.tile([P, NK * WID], F8, tag="m8")
            m8g = m8[:].rearrange("p (g w) -> p g w", w=W)      # g = ki*NT+tt
            for ki in range(NK):
                nc.gpsimd.dma_start(
                    m8g[:, ki * NT : (ki + 1) * NT, :], _wide(m8d[ki]))
            sc = []
            for c in range(C):
                t = sb.tile([P, WID], F16, tag=f"s{c}")
                for hlf in range(2):
                    nc.gpsimd.dma_start(
                        _w3(t[:])[:, 2 * hlf : 2 * hlf + 2, :],
                        _wide(score[c])[:, 2 * hlf : 2 * hlf + 2, :])
                sc.append(t)
            # ---- accumulator tile ----
            acc = sb.tile([P, NACC], F32, tag="acc")
            nc.vector.memset(acc[:], 0.0)

            # ---- spins from int8 label (f16); class counts done on host ----
            a01 = sb.tile([P, WID], I8, tag="a01")
            nc.vector.tensor_scalar(a01[:], lbl[:], 2, None, OP.is_ge)
            sa = sb.tile([P, WID], F16, tag="sa")
            nc.vector.tensor_scalar(sa[:], a01[:], -2.0, 1.0, OP.mult, OP.add)
            b01 = sb.tile([P, WID], I8, tag="b01")
            nc.vector.tensor_scalar(b01[:], lbl[:], 1, None, OP.bitwise_and)
            sbn = sb.tile([P, WID], F16, tag="sbn")
            nc.vector.tensor_scalar(sbn[:], b01[:], -2.0, 1.0, OP.mult, OP.add)
            sab = sb.tile([P, WID], F16, tag="sab")
            nc.vector.tensor_mul(sab[:], sa[:], sbn[:])
            spins = [sa, sbn, sab]

            # class masks for the s_label gather (vector, off critical path)
            masks = []
            for c in range(1, C):
                m = sb.tile([P, WID], I8, tag=f"msk{c}")
                nc.vector.tensor_scalar(m[:], lbl[:], float(c), None,
                                        OP.is_equal)
                masks.append(m)

            # ---- transposed spins via DMA xbar (4 xbar calls per map) ----
            spT = []
            for mi, sp in enumerate(spins):
                t = sb.tile([P, WID], F16, tag=f"spT{mi}")
                t3 = _w3(t[:])
                sp3 = _w3(sp[:])
                for th in range(NT):
                    nc.sync.dma_start_transpose(
                        t3[:, :, th * P : (th + 1) * P], sp3[:, th, :])
                spT.append(t)

            # ---- per-map pipeline, PE-dense ordering:
            #      pass1 x3 back-to-back (keeps the PE p-state ramped), dv
            #      builds overlap later pass1s, then pass2 x3 fp8 DR ----
            gps = ps.tile([P, WID], F32, tag="ps_g", bufs=1)
            vtile = sb.tile([P, WID], F16, tag="vtile")
            cvts = []

            def do_pass1(mi, sp):
                # pass1: Cv^T[w, h'] = sum_h sp[h,w] U[h,h'] (triangular)
                cvt = sb.tile([P, NT * CVW], F16, tag="cvt", bufs=3)
                cv3 = cvt[:].rearrange("p (t w) -> p t w", t=NT)
                cvts.append(cv3)
                for j in range(NT):  # w-chunk -> psum partitions
                    pst = ps.tile([P, W], F32, tag="ps_cv", bufs=2)
                    for tt in range(NT):  # contraction over h-tiles
                        nc.tensor.matmul(
                            pst[:, P * tt : W],
                            sp[:, W * tt + P * j : W * tt + P * j + P],
                            u16[:, W * tt + P * tt : W * tt + W],
                            start=(tt == 0),
                            stop=(tt == NT - 1),
                            skip_group_check=True,
                        )
                    nc.scalar.copy(cv3[:, j, LP : LP + W], pst[:])
                # pads: left zeros, right replicate Cv(511)
                nc.vector.memset(cv3[:, :, 0:LP], 0.0)
                nc.vector.tensor_copy(
                    cv3[:, :, LP + W : CVW],
                    cv3[:, :, LP + W - 1 : LP + W].broadcast_to([P, NT, LP]))

            def do_dv(mi):
                """f16 shift-diff subs (2x mode), then f16->f8 converts
                split between the scalar and vector engines."""
                cv3 = cvts[mi]
                dv8 = sb.tile([P, NK * WID], F8, tag="dv8", bufs=2)
                dv8g = dv8[:].rearrange("p (g w) -> p g w", w=W)  # g=ki*NT+tt
                for ki in range(NK):
                    p = PADS[ki]
                    d16 = sb.tile([P, WID], F16, tag="dv16", bufs=3)
                    nc.vector.tensor_sub(
                        _w3(d16[:]),
                        cv3[:, :, LP + p : LP + p + W],
                        cv3[:, :, LP - p - 1 : LP - p - 1 + W],
                    )
                    dst = dv8g[:, ki * NT : (ki + 1) * NT, :].rearrange(
                        "p g w -> p (g w)")
                    if ki % 2 == 0:
                        nc.scalar.copy(dst, d16[:])
                    else:
                        nc.vector.tensor_scalar(dst, d16[:], 1.0, None,
                                                OP.mult)
                return dv8g

            def do_mm(mi, dv8g):
                # pass2: G^T chunks via fp8 DoubleRow band matmuls
                for j in range(NT):
                    items = [(ki, tt) for ki in range(NK)
                             for tt in (j - 1, j, j + 1) if 0 <= tt < NT]
                    prs = [(items[2 * i], items[2 * i + 1])
                           for i in range(len(items) // 2)]
                    single = items[-1] if len(items) % 2 else None
                    n_ins = len(prs) + (1 if single else 0)
                    for idx, ((ka, ta), (kb, tbt)) in enumerate(prs):
                        ga, gb = ka * NT + ta, kb * NT + tbt
                        d = gb - ga
                        nc.tensor.matmul(
                            gps[:, W * j : W * (j + 1)],
                            m8g[:, ga : gb + 1 : d, P * j : P * j + P],
                            dv8g[:, ga : gb + 1 : d, :],
                            start=(idx == 0),
                            stop=(idx == n_ins - 1),
                            perf_mode=DR,
                            skip_group_check=True,
                        )
                    if single:
                        ks_, ts_ = single
                        gsi = ks_ * NT + ts_
                        nc.tensor.matmul(
                            gps[:, W * j : W * (j + 1)],
                            m8g[:, gsi, P * j : P * j + P],
                            dv8g[:, gsi, :],
                            start=False,
                            stop=True,
                            skip_group_check=True,
                        )

            def do_pass2(mi, unused=None):
                # evacuate G^T per j-chunk on scalar with the 2^-SC fold
                # (activation Copy with scale), then v_m and pix += v_m
                gt = sb.tile([P, WID], F16, tag="gt", bufs=2)
                for j in range(NT):
                    if mi == 2 and j >= 2:
                        nc.vector.tensor_scalar(
                            gt[:, W * j : W * (j + 1)],
                            gps[:, W * j : W * (j + 1)],
                            2.0**-SC, None, OP.mult)
                    else:
                        nc.scalar.activation(gt[:, W * j : W * (j + 1)],
                                             gps[:, W * j : W * (j + 1)],
                                             ACTF.Copy, scale=2.0**-SC)
                if debug:
                    gtd = sb.tile([P, WID], F32, tag="gtd")
                    nc.vector.tensor_scalar(gtd[:], gt[:], 1.0, None, OP.mult)
                    nc.sync.dma_start(_wide(gt_dbg[mi]), _w3(gtd[:]))
                nc.vector.tensor_mul(vtile[:], spT[mi][:], gt[:])
                nc.vector.tensor_add(pix[:], pix[:], vtile[:])

            # PE-dense schedule with the scalar-engine CE exps slotted where
            # they do not stall either pipeline (sequencers are in-order)
            do_pass1(0, spins[0])
            do_pass1(1, spins[1])
            d0 = do_dv(0)
            do_pass1(2, spins[2])
            do_mm(0, d0)
            do_pass2(0)
            d1 = do_dv(1)
            do_mm(1, d1)
            ex = []
            for c in range(2):
                t = sb.tile([P, WID], F16, tag=f"ex{c}")
                nc.scalar.activation(t[:], sc[c][:], ACTF.Exp)
                ex.append(t)
            do_pass2(1)
            for c in range(2, C):
                t = sb.tile([P, WID], F16, tag=f"ex{c}")
                nc.scalar.activation(t[:], sc[c][:], ACTF.Exp)
                ex.append(t)
            # exp sums on vector while map2 runs
            s01 = ex[0]
            nc.vector.tensor_add(s01[:], ex[0][:], ex[1][:])
            nc.vector.tensor_add(ex[2][:], ex[2][:], ex[3][:])
            nc.vector.tensor_add(s01[:], s01[:], ex[2][:])
            lse = ex[1]  # reuse
            nc.scalar.activation(lse[:], s01[:], ACTF.Ln)
            # s_label gather then nll = lse - sl
            sl = ex[3]  # reuse
            nc.vector.tensor_copy(sl[:], sc[0][:])
            for ci in range(3):
                nc.vector.copy_predicated(sl[:], masks[ci][:], sc[ci + 1][:])
            nll = sc[0]  # reuse
            nc.vector.tensor_sub(nll[:], lse[:], sl[:])
            if debug:
                nc.sync.dma_start(_wide(nll_dbg[:]), _w3(nll[:]))
            # nll^T via DMA xbar (sync seq; scalar is busy with evacs/exps)
            nllt = sc[1]  # reuse
            nllt3 = _w3(nllt[:])
            nll3 = _w3(nll[:])
            for th in range(NT):
                eng = nc.sync if th % 2 == 0 else nc.scalar
                eng.dma_start_transpose(
                    nllt3[:, :, th * P : (th + 1) * P], nll3[:, th, :])
            d2 = do_dv(2)
            do_mm(2, d2)
            do_pass2(2)

            if debug:
                nc.sync.dma_start(
                    pix_dbg[:].rearrange("(t p) h -> p t h", p=P), _w3(pix[:]))

            # ---- T reductions: q = nll^T * pix^T, then per-spin accums
            #      (accumulating TensorScalarPtr is vector-only) ----
            q = sc[2]    # reuse
            junk = sc[3]  # reuse
            nc.vector.scalar_tensor_tensor(
                q[:], nllt[:], 1.0, pix[:],
                OP.mult, OP.mult, accum_out=acc[:, A_T0 : A_T0 + 1])
            nc.sync.dma_start(accd[:, A_T0 : A_T0 + 1],
                              acc[:, A_T0 : A_T0 + 1])
            for mi in range(3):
                vcol = (A_TA, A_TBQ, A_TABQ)[mi]
                nc.vector.scalar_tensor_tensor(
                    junk[:], q[:], 1.0, spT[mi][:],
                    OP.mult, OP.mult, accum_out=acc[:, vcol : vcol + 1])

            nc.sync.dma_start(accd[:, A_TA:NACC], acc[:, A_TA:NACC])

    nc.finalize()
    return nc


_CACHE = {}


def _get_nc(debug=False):
    key = "dbg" if debug else "fast"
    if key not in _CACHE:
        _CACHE[key] = build_nc(debug)
    return _CACHE[key]


def run_cores(cls_score, label, debug=False, trace=False):
    """Run the SPMD kernel; returns BassKernelResults."""
    U, M8, CON = _host_constants()
    score16 = cls_score.astype(np.float16)
    lbl8 = label.astype(np.int8)
    in_maps = []
    for i in range(N_CORES):
        in_maps.append(
            {
                "score": np.ascontiguousarray(score16[i]),
                "label": np.ascontiguousarray(lbl8[i]),
                "u16": U,
                "m8": M8,
                "con": CON,
            }
        )
    nc = _get_nc(debug)
    return run_bass_kernel_spmd(nc, in_maps, list(range(N_CORES)), trace=trace)


def kernel(cls_score, label):
    cls_score = np.asarray(cls_score, dtype=np.float32)
    label = np.asarray(label, dtype=np.int32)
    res = run_cores(cls_score, label)
    A = np.zeros(NACC, dtype=np.float64)
    for r in res.results:
        A += r["acc"].astype(np.float64).sum(axis=0)
    npix = float(B * H * W)
    # N-sums from host-side class counts (cheap vs device accumulators)
    cnt = np.bincount(label.ravel(), minlength=C).astype(np.float64)
    Na = (cnt[0] + cnt[1]) - (cnt[2] + cnt[3])
    Nb = (cnt[0] + cnt[2]) - (cnt[1] + cnt[3])
    Nab = (cnt[0] + cnt[3]) - (cnt[1] + cnt[2])
    N = [Na, Nb, Nab]
    T = [A[A_T0], A[A_TA], A[A_TBQ], A[A_TABQ]]
    loss = 0.0
    for c in range(C):
        sig_a = 1.0 - 2.0 * (c >> 1)
        sig_b = 1.0 - 2.0 * (c & 1)
        n_c = 0.25 * (npix + sig_a * N[0] + sig_b * N[1] + sig_a * sig_b * N[2])
        s_c = 0.25 * (T[0] + sig_a * T[1] + sig_b * T[2] + sig_a * sig_b * T[3])
        w_c = 2.0 / (n_c / npix + 1.0)
        loss += w_c * s_c
    return np.float32(loss / npix)


if __name__ == "__main__":
    rng = np.random.default_rng(0)
    cs = rng.standard_normal((B, C, H, W)).astype(np.float32)
    lb = rng.integers(0, C, size=(B, H, W)).astype(np.int32)
    print("loss:", kernel(cs, lb))
